# revision 2
# baseline (speedup 1.0000x reference)
"""Trainium2 Bass kernel for nn_CatEncoderCross.

Computes out[b,i,j,:] = input1[b,i,:] @ W[:768] + input2[b,j,:] @ W[768:] + bias
for shapes input1/input2 [4,128,768], W [1536,768], b [768],
output [4,128,128,768] (f32, ~192 MB).

Sharding: data-parallel over (batch, n1-half): core c handles batch c//2,
n1 rows [64*(c%2), 64*(c%2)+64).  Per core:
  p1 = x1_shard @ W1          [64, 768]   (PE, fp32)
  p2 = x2_batch @ W2 + bias   [128, 768]  (PE, fp32)
  for each i: psum = ones ⊗ p1[i]  (K=1 broadcast matmul, f32r)
              out_tile = psum + p2 (DVE tensor_tensor)
              DMA out_tile -> HBM
"""

import os
import numpy as np

P = 128
D = 768
KO = 6  # K chunks of 128 in d1 (=d2)
NI = 64  # n1 rows per core
NJ = 128  # n2
NCORES = 8

# matmul variant for the broadcast matmuls: "f32r" (fast) or "f32" (safe)
BCAST_DTYPE = os.environ.get("KERNEL_BCAST_DTYPE", "f32r")
# matmul variant for the p1/p2 projection matmuls
PROJ_DTYPE = os.environ.get("KERNEL_PROJ_DTYPE", "f32")
# output tiles batched per DMA
TILES_PER_DMA = int(os.environ.get("KERNEL_TILES_PER_DMA", "2"))

_cache = {}


def _build_module():
    import concourse.bass as bass
    import concourse.bacc as bacc
    import concourse.mybir as mybir
    import concourse.tile as tile

    F32 = mybir.dt.float32
    F32R = mybir.dt.float32r

    nc = bacc.Bacc("TRN2", target_bir_lowering=False, debug=False)

    x1T_d = nc.dram_tensor("x1T", [P, KO, NI], F32, kind="ExternalInput")
    x2T_d = nc.dram_tensor("x2T", [P, KO, NJ], F32, kind="ExternalInput")
    w_d = nc.dram_tensor("Wr", [P, 2 * KO, D], F32, kind="ExternalInput")
    bias_d = nc.dram_tensor("biasr", [1, D], F32, kind="ExternalInput")
    out_d = nc.dram_tensor("out", [NI, NJ, D], F32, kind="ExternalOutput")
    out_ap = out_d.ap()

    def mm_cast(ap):
        if BCAST_DTYPE == "f32r":
            return ap.bitcast(F32R)
        return ap

    def proj_cast(ap):
        if PROJ_DTYPE == "f32r":
            return ap.bitcast(F32R)
        return ap

    with tile.TileContext(nc) as tc:
        with (
            tc.tile_pool(name="const", bufs=1) as cpool,
            tc.tile_pool(name="psum", bufs=3, space="PSUM") as pspool,
            tc.tile_pool(name="outp", bufs=3) as opool,
        ):
            w_sb = [cpool.tile([P, D], F32, tag=f"w{o}", name=f"w{o}") for o in range(2 * KO)]
            x1T_sb = cpool.tile([P, KO, NI], F32, tag="x1T")
            x2T_sb = cpool.tile([P, KO, NJ], F32, tag="x2T")
            bias_sb = cpool.tile([1, D], F32, tag="bias")
            ones_sb = cpool.tile([33, P], F32, tag="ones")
            p1_sb = cpool.tile([NI, D], F32, tag="p1")
            p2_sb = cpool.tile([P, D], F32, tag="p2")
            # p1 rows flattened into the free dim: rows 0-31 on partition 0,
            # rows 32-63 on partition 32 (matmul base_partition must be 0/32/64)
            p1f = cpool.tile([33, 32, D], F32, tag="p1f")

            nc.vector.memset(ones_sb[:], 1.0)

            # --- input DMAs (W1 chunks first: p1 is on the critical path) ---
            nc.sync.dma_start(out=x1T_sb[:], in_=x1T_d.ap())
            nc.sync.dma_start(out=x2T_sb[:], in_=x2T_d.ap())
            nc.sync.dma_start(out=bias_sb[:], in_=bias_d.ap())
            for o in range(2 * KO):
                nc.sync.dma_start(out=w_sb[o][:], in_=w_d.ap()[:, o, :])

            # --- p1 = x1 @ W1 ---
            p1_ps_t = pspool.tile([P, 1024], F32, tag="ps")
            p1_ps = p1_ps_t[:NI]
            for o in range(KO):
                nc.tensor.matmul(
                    p1_ps[:, 0:512],
                    proj_cast(x1T_sb[:, o, :]),
                    proj_cast(w_sb[o][:, 0:512]),
                    start=(o == 0),
                    stop=(o == KO - 1),
                )
                nc.tensor.matmul(
                    p1_ps[:, 512:D],
                    proj_cast(x1T_sb[:, o, :]),
                    proj_cast(w_sb[o][:, 512:D]),
                    start=(o == 0),
                    stop=(o == KO - 1),
                )
            nc.vector.tensor_copy(out=p1_sb[:], in_=p1_ps[:, 0:D])

            # flatten p1 rows into per-partition free dim (SBUF->SBUF DMA)
            nc.sync.dma_start(out=p1f[0:1], in_=p1_sb[0:32, :])
            nc.sync.dma_start(out=p1f[32:33], in_=p1_sb[32:64, :])

            # --- p2 = x2 @ W2 + bias ---
            p2_ps_t = pspool.tile([P, 1024], F32, tag="ps")
            p2_ps = p2_ps_t
            for o in range(KO):
                nc.tensor.matmul(
                    p2_ps[:, 0:512],
                    proj_cast(x2T_sb[:, o, :]),
                    proj_cast(w_sb[KO + o][:, 0:512]),
                    start=(o == 0),
                    stop=False,
                )
                nc.tensor.matmul(
                    p2_ps[:, 512:D],
                    proj_cast(x2T_sb[:, o, :]),
                    proj_cast(w_sb[KO + o][:, 512:D]),
                    start=(o == 0),
                    stop=False,
                )
            # bias via K=1 ones matmul, accumulated on top
            nc.tensor.matmul(
                p2_ps[:, 0:512],
                proj_cast(ones_sb[0:1, :]),
                proj_cast(bias_sb[:, 0:512]),
                start=False,
                stop=True,
            )
            nc.tensor.matmul(
                p2_ps[:, 512:D],
                proj_cast(ones_sb[0:1, :]),
                proj_cast(bias_sb[:, 512:D]),
                start=False,
                stop=True,
            )
            nc.vector.tensor_copy(out=p2_sb[:], in_=p2_ps[:, 0:D])

            # --- main loop: broadcast p1[i] over 128 partitions, add p2 ---
            TPD = TILES_PER_DMA
            ob = None
            for i in range(NI):
                h, r = divmod(i, 32)
                if i % TPD == 0:
                    ob = opool.tile([P, TPD, D], F32, tag="ob", name=f"ob{i}")
                ps = pspool.tile([P, 1024], F32, tag="ps", name=f"ps{i}")
                lhsT = mm_cast(ones_sb[32 * h : 32 * h + 1, :])
                rhs = mm_cast(p1f[32 * h : 32 * h + 1, r, :])
                nc.tensor.matmul(
                    ps[:, 0:512], lhsT, rhs[:, 0:512], start=True, stop=True
                )
                nc.tensor.matmul(
                    ps[:, 512:D], lhsT, rhs[:, 512:D], start=True, stop=True
                )
                nc.vector.tensor_add(
                    out=ob[:, i % TPD, :], in0=ps[:, 0:D], in1=p2_sb[:]
                )
                if i % TPD == TPD - 1:
                    i0 = i - (TPD - 1)
                    dst = out_ap[i0 : i + 1]  # [TPD, NJ, D]
                    nc.sync.dma_start(
                        out=dst.rearrange("i j d -> j i d"), in_=ob[:]
                    )

    nc.compile()
    return nc


def _get_module():
    key = (BCAST_DTYPE, PROJ_DTYPE, TILES_PER_DMA)
    if key not in _cache:
        _cache[key] = _build_module()
    return _cache[key]


def _make_in_maps(input1, input2, W, b):
    input1 = np.asarray(input1, dtype=np.float32)
    input2 = np.asarray(input2, dtype=np.float32)
    W = np.asarray(W, dtype=np.float32)
    b = np.asarray(b, dtype=np.float32)

    Wr = np.ascontiguousarray(W.reshape(2 * KO, P, D).transpose(1, 0, 2))
    biasr = np.ascontiguousarray(b.reshape(1, D))
    in_maps = []
    for c in range(NCORES):
        bb, h = divmod(c, 2)
        x1 = input1[bb, h * NI : (h + 1) * NI]  # [64, 768]
        x2 = input2[bb]  # [128, 768]
        x1T = np.ascontiguousarray(x1.T.reshape(KO, P, NI).transpose(1, 0, 2))
        x2T = np.ascontiguousarray(x2.T.reshape(KO, P, NJ).transpose(1, 0, 2))
        in_maps.append({"x1T": x1T, "x2T": x2T, "Wr": Wr, "biasr": biasr})
    return in_maps


def kernel(input1, input2, W, b):
    from concourse import bass_utils

    nc = _get_module()
    in_maps = _make_in_maps(input1, input2, W, b)
    res = bass_utils.run_bass_kernel_spmd(
        nc, in_maps, core_ids=list(range(NCORES))
    )
    out = np.empty((4, NJ, NJ, D), dtype=np.float32)
    for c in range(NCORES):
        bb, h = divmod(c, 2)
        out[bb, h * NI : (h + 1) * NI] = res.results[c]["out"]
    return out


# revision 18
# speedup vs baseline: 1.1552x; 1.1552x over previous
"""Trainium2 Bass kernel for nn_CatEncoderCross.

Computes out[b,i,j,:] = input1[b,i,:] @ W[:768] + input2[b,j,:] @ W[768:] + bias
for shapes input1/input2 [4,128,768], W [1536,768], b [768],
output [4,128,128,768] (f32, ~192 MB).

Sharding: data-parallel over (batch, n1-half): core c handles batch c//2,
n1 rows [64*(c%2), 64*(c%2)+64).  Per core:
  p1 = x1_shard @ W1          [64, 768]   (PE)
  p2 = x2_batch @ W2 + bias   [128, 768]  (PE)
  for each i: psum = ones ⊗ p1[i]  (K-stacked broadcast matmul)
              out_tile = psum + p2 (DVE tensor_tensor)
              DMA out_tile -> HBM

Precision modes:
  f32    - native fp32 matmul (4 cyc/row, slowest, exact)
  f32r   - fp32 "replicated" single-pass matmul (1 cyc/row, ~1e-4 rel err)
  bf16hl - split fp32 into bf16 hi+lo; contract both (1 cyc/row, ~1e-5 rel err)
"""

import os
import numpy as np

P = 128
D = 768
KO = 6  # K chunks of 128 in d1 (=d2)
NI = 64  # n1 rows per core
NJ = 128  # n2
NCORES = 8

BCAST_DTYPE = os.environ.get("KERNEL_BCAST_DTYPE", "bf16hl")
PROJ_DTYPE = os.environ.get("KERNEL_PROJ_DTYPE", "bf16hl")
TILES_PER_DMA = int(os.environ.get("KERNEL_TILES_PER_DMA", "2"))
PSUM_BUFS = int(os.environ.get("KERNEL_PSUM_BUFS", "2"))
OUT_BUFS = int(os.environ.get("KERNEL_OUT_BUFS", "3"))
# perf-probe only: comma list of stages to skip (never set in real runs)
SKIP = set(s for s in os.environ.get("KERNEL_SKIP", "").split(",") if s)
# fuse pairs of output tiles into one DVE op (psum tiles span 4 banks)
FUSE_PAIR = os.environ.get("KERNEL_FUSE_PAIR", "1") == "1"
# PE warm-up matmuls issued while weights stream in (HAM ramp to 2.4 GHz)
WARM_MMS = int(os.environ.get("KERNEL_WARM_MMS", "30"))
# consolidate W load into 2 big DMAs (one per ring) instead of per-chunk
BIG_W_DMA = os.environ.get("KERNEL_BIG_W_DMA", "0") == "1"
# alternate output DMAs between the two HWDGE rings (sync/scalar)
ALT_OUT_RING = os.environ.get("KERNEL_ALT_OUT_RING", "0") == "1"
# W chunk-pairs per DMA (1, 2, 3, or 6)
W_GROUP = int(os.environ.get("KERNEL_W_GROUP", "1"))
# load x/bias via SWDGE (gpsimd) so the sync ring is all-W
X_ON_SWDGE = os.environ.get("KERNEL_X_ON_SWDGE", "0") == "1"

_cache = {}


def _split_hl(x):
    """Split fp32 array into bf16 hi + lo with x ~= hi + lo."""
    import ml_dtypes

    hi = x.astype(ml_dtypes.bfloat16)
    lo = (x - hi.astype(np.float32)).astype(ml_dtypes.bfloat16)
    return hi, lo


def _build_module():
    import concourse.bacc as bacc
    import concourse.mybir as mybir
    import concourse.tile as tile

    F32 = mybir.dt.float32
    F32R = mybir.dt.float32r
    BF16 = mybir.dt.bfloat16

    nc = bacc.Bacc("TRN2", target_bir_lowering=False, debug=False)

    # --- DRAM I/O ---
    proj_hl = PROJ_DTYPE == "bf16hl"
    xdt = BF16 if proj_hl else F32
    nx1 = 2 if proj_hl else 1  # hi/lo planes
    x1T_d = nc.dram_tensor("x1T", [P, nx1 * KO, NI], xdt, kind="ExternalInput")
    x2T_d = nc.dram_tensor("x2T", [P, nx1 * KO, NJ], xdt, kind="ExternalInput")
    w_d = nc.dram_tensor("Wr", [P, nx1 * 2 * KO, D], xdt, kind="ExternalInput")
    bias_d = nc.dram_tensor("biasr", [2, D], BF16, kind="ExternalInput")
    out_d = nc.dram_tensor("out", [NI, NJ, D], F32, kind="ExternalOutput")
    out_ap = out_d.ap()

    def mm_cast(ap):
        return ap.bitcast(F32R) if BCAST_DTYPE == "f32r" else ap

    def proj_cast(ap):
        return ap.bitcast(F32R) if PROJ_DTYPE == "f32r" else ap

    with tile.TileContext(nc) as tc:
        with (
            tc.tile_pool(name="const", bufs=1) as cpool,
            tc.tile_pool(name="psum", bufs=PSUM_BUFS, space="PSUM") as pspool,
            tc.tile_pool(name="outp", bufs=OUT_BUFS) as opool,
        ):
            G = W_GROUP
            assert KO % G == 0
            w_grp = [
                cpool.tile([P, G, nx1, D], xdt, tag=f"w{g}", name=f"w{g}")
                for g in range(2 * KO // G)
            ]
            w_sb = [
                w_grp[o // (nx1 * G)][:, (o // nx1) % G, o % nx1]
                for o in range(nx1 * 2 * KO)
            ]
            x1T_sb = cpool.tile([P, nx1 * KO, NI], xdt, tag="x1T")
            x2T_sb = cpool.tile([P, nx1 * KO, NJ], xdt, tag="x2T")
            bias_sb = cpool.tile([2, D], BF16, tag="bias")
            ones_sb = cpool.tile([34, P], F32, tag="ones")
            ones_bf = cpool.tile([34, P], BF16, tag="ones_bf")
            p1_sb = cpool.tile([NI, D], F32, tag="p1")
            p2_sb = cpool.tile([P, D], F32, tag="p2")

            nc.vector.memset(ones_sb[:], 1.0)
            nc.vector.memset(ones_bf[:], 1.0)

            psw = 2048 if FUSE_PAIR else 1024
            if WARM_MMS:
                warm_ps = pspool.tile([P, psw], F32, tag="ps", name="warm_ps")
                for wi in range(WARM_MMS):
                    nc.tensor.matmul(
                        warm_ps[:, 0:P],
                        ones_bf[0:2, :],
                        ones_bf[0:2, :],
                        start=True,
                        stop=True,
                    )

            # --- input DMAs (x1T then W1 chunks first: p1 gates the
            # broadcast pipeline; x2T/bias slot in before the W2 chunks) ---
            wr = w_d.ap().rearrange("p (g o x) d -> p g o x d", x=nx1, o=G)
            xeng = nc.gpsimd if X_ON_SWDGE else nc.sync
            ngrp = KO // G
            xeng.dma_start(out=x1T_sb[:], in_=x1T_d.ap())
            for g in range(ngrp):
                nc.sync.dma_start(out=w_grp[g][:], in_=wr[:, g])
            xeng.dma_start(out=x2T_sb[:], in_=x2T_d.ap())
            xeng.dma_start(out=bias_sb[:], in_=bias_d.ap())
            for g in range(ngrp, 2 * ngrp):
                nc.sync.dma_start(out=w_grp[g][:], in_=wr[:, g])

            nhalf = [(0, 512), (512, D)]

            def proj_matmuls(ps, x_sb, w_off, m, extra_tail):
                """ps[:m, :768] = x @ W_half  (+ tail MMs continue the group)"""
                if proj_hl:
                    # per k-chunk: xh@Wh, xh@Wl, xl@Wh  (W planes interleaved
                    # h,l per chunk; x planes: [0:KO]=hi, [KO:2KO]=lo)
                    steps = []
                    for o in range(KO):
                        steps.append((o, 2 * (w_off + o) + 0))  # xh @ Wh
                        steps.append((o, 2 * (w_off + o) + 1))  # xh @ Wl
                        steps.append((KO + o, 2 * (w_off + o) + 0))  # xl @ Wh
                else:
                    steps = [(o, w_off + o) for o in range(KO)]
                for si, (xo, wo) in enumerate(steps):
                    for lo_, hi_ in nhalf:
                        nc.tensor.matmul(
                            ps[:m, lo_:hi_],
                            proj_cast(x_sb[:, xo, :]),
                            proj_cast(w_sb[wo][:, lo_:hi_]),
                            start=(si == 0),
                            stop=(si == len(steps) - 1 and not extra_tail),
                        )

            # --- p1 = x1 @ W1 ---
            do_proj = "proj" not in SKIP
            if do_proj:
                p1_ps_t = pspool.tile([P, psw], F32, tag="ps", name="p1_ps")
                proj_matmuls(p1_ps_t, x1T_sb, 0, NI, extra_tail=False)
                nc.vector.tensor_copy(out=p1_sb[:], in_=p1_ps_t[:NI, 0:D])

            # --- p2 = x2 @ W2 + bias ---
            if do_proj:
                p2_ps = pspool.tile([P, psw], F32, tag="ps", name="p2_ps")
                proj_matmuls(p2_ps, x2T_sb, KO, P, extra_tail=True)
            if do_proj:
                # bias via K=2 ones matmul on bf16 hi/lo rows (exact to ~1e-5)
                for lo_, hi_ in nhalf:
                    nc.tensor.matmul(
                        p2_ps[:, lo_:hi_],
                        ones_bf[0:2, :],
                        bias_sb[:, lo_:hi_],
                        start=False,
                        stop=(hi_ == D),
                    )
                nc.vector.tensor_copy(out=p2_sb[:], in_=p2_ps[:, 0:D])

            # --- p1 broadcast source layout ---
            bcast_hl = BCAST_DTYPE == "bf16hl"
            if not do_proj:
                pass
            elif bcast_hl:
                p1h_sb = cpool.tile([NI, D], BF16, tag="p1h")
                p1l_sb = cpool.tile([NI, D], BF16, tag="p1l")
                nc.vector.tensor_copy(out=p1h_sb[:], in_=p1_sb[:])
                nc.vector.tensor_sub(
                    out=p1l_sb[:], in0=p1_sb[:], in1=p1h_sb[:]
                )
                # rows r<32 -> partitions 0(hi),1(lo); r>=32 -> 32(hi),33(lo)
                p1f = cpool.tile([34, 32, D], BF16, tag="p1f")
                nc.sync.dma_start(out=p1f[0:1], in_=p1h_sb[0:32, :])
                nc.sync.dma_start(out=p1f[1:2], in_=p1l_sb[0:32, :])
                nc.sync.dma_start(out=p1f[32:33], in_=p1h_sb[32:64, :])
                nc.sync.dma_start(out=p1f[33:34], in_=p1l_sb[32:64, :])
            else:
                p1f = cpool.tile([33, 32, D], F32, tag="p1f")
                nc.sync.dma_start(out=p1f[0:1], in_=p1_sb[0:32, :])
                nc.sync.dma_start(out=p1f[32:33], in_=p1_sb[32:64, :])
            if not do_proj:
                pdt = BF16 if bcast_hl else F32
                p1f = cpool.tile([34, 32, D], pdt, tag="p1f")

            # --- main loop ---
            def bcast_ops(i):
                h = i // 32
                r = i % 32
                if bcast_hl:
                    return ones_bf[32 * h : 32 * h + 2, :], p1f[32 * h : 32 * h + 2, r, :]
                return (
                    mm_cast(ones_sb[32 * h : 32 * h + 1, :]),
                    mm_cast(p1f[32 * h : 32 * h + 1, r, :]),
                )

            if FUSE_PAIR:
                TPD = TILES_PER_DMA
                assert TPD % 2 == 0
                ob = None
                for pair in range(NI // 2):
                    i0 = 2 * pair
                    if i0 % TPD == 0:
                        ob = opool.tile([P, TPD, D], F32, tag="ob", name=f"ob{pair}")
                    ps = pspool.tile([P, 2048], F32, tag="ps", name=f"ps{pair}")
                    if "mm1" not in SKIP:
                        for m in range(2):
                            lhsT, rhs = bcast_ops(i0 + m)
                            for lo_, hi_ in nhalf:
                                nc.tensor.matmul(
                                    ps[:, 1024 * m + lo_ : 1024 * m + hi_],
                                    lhsT,
                                    rhs[:, lo_:hi_],
                                    start=True,
                                    stop=True,
                                )
                    if "add" not in SKIP:
                        ps_v = ps.rearrange("p (i x) -> p i x", i=2)[:, :, 0:D]
                        p2_b = p2_sb[:, None, :].to_broadcast((P, 2, D))
                        ot = i0 % TPD
                        nc.vector.tensor_add(
                            out=ob[:, ot : ot + 2, :], in0=ps_v, in1=p2_b
                        )
                    if "dmaout" not in SKIP and (i0 + 2) % TPD == 0:
                        j0 = i0 + 2 - TPD
                        dst = out_ap[j0 : i0 + 2]  # [TPD, NJ, D]
                        eng = nc.scalar if (ALT_OUT_RING and (j0 // TPD) % 2) else nc.sync
                        eng.dma_start(
                            out=dst.rearrange("i j d -> j i d"), in_=ob[:]
                        )
            else:
                TPD = TILES_PER_DMA
                ob = None
                for i in range(NI):
                    if i % TPD == 0:
                        ob = opool.tile([P, TPD, D], F32, tag="ob", name=f"ob{i}")
                    ps = pspool.tile([P, 1024], F32, tag="ps", name=f"ps{i}")
                    lhsT, rhs = bcast_ops(i)
                    if "mm1" not in SKIP:
                        for lo_, hi_ in nhalf:
                            nc.tensor.matmul(
                                ps[:, lo_:hi_],
                                lhsT,
                                rhs[:, lo_:hi_],
                                start=True,
                                stop=True,
                            )
                    if "add" not in SKIP:
                        nc.vector.tensor_add(
                            out=ob[:, i % TPD, :], in0=ps[:, 0:D], in1=p2_sb[:]
                        )
                    if "dmaout" not in SKIP and i % TPD == TPD - 1:
                        i0 = i - (TPD - 1)
                        dst = out_ap[i0 : i + 1]  # [TPD, NJ, D]
                        nc.sync.dma_start(
                            out=dst.rearrange("i j d -> j i d"), in_=ob[:]
                        )

    nc.compile()
    return nc


def _variant_key():
    return (
        BCAST_DTYPE,
        PROJ_DTYPE,
        TILES_PER_DMA,
        PSUM_BUFS,
        OUT_BUFS,
        tuple(sorted(SKIP)),
        FUSE_PAIR,
        WARM_MMS,
        BIG_W_DMA,
        ALT_OUT_RING,
        W_GROUP,
        X_ON_SWDGE,
    )


def _get_module():
    key = _variant_key()
    if key not in _cache:
        _cache[key] = _build_module()
    return _cache[key]


def _prep_xT(x, n):
    """[rows, 768] -> [128, KO, rows] transposed chunk layout."""
    return np.ascontiguousarray(x.T.reshape(KO, P, n).transpose(1, 0, 2))


def _make_in_maps(input1, input2, W, b):
    input1 = np.asarray(input1, dtype=np.float32)
    input2 = np.asarray(input2, dtype=np.float32)
    W = np.asarray(W, dtype=np.float32)
    b = np.asarray(b, dtype=np.float32)

    proj_hl = PROJ_DTYPE == "bf16hl"
    if proj_hl:
        Wh, Wl = _split_hl(W)
        # interleave planes per k-chunk: Wr[:, 2o+0]=Wh chunk, 2o+1=Wl chunk
        Wrh = Wh.reshape(2 * KO, P, D).transpose(1, 0, 2)
        Wrl = Wl.reshape(2 * KO, P, D).transpose(1, 0, 2)
        Wr = np.ascontiguousarray(
            np.stack([Wrh, Wrl], axis=2).reshape(P, 4 * KO, D)
        )
    else:
        Wr = np.ascontiguousarray(W.reshape(2 * KO, P, D).transpose(1, 0, 2))
    bh, bl = _split_hl(b)
    biasr = np.ascontiguousarray(np.stack([bh, bl], axis=0))  # bf16 hi/lo rows

    in_maps = []
    for c in range(NCORES):
        bb, h = divmod(c, 2)
        x1 = input1[bb, h * NI : (h + 1) * NI]  # [64, 768]
        x2 = input2[bb]  # [128, 768]
        if proj_hl:
            x1h, x1l = _split_hl(x1)
            x2h, x2l = _split_hl(x2)
            x1T = np.ascontiguousarray(
                np.concatenate([_prep_xT(x1h, NI), _prep_xT(x1l, NI)], axis=1)
            )
            x2T = np.ascontiguousarray(
                np.concatenate([_prep_xT(x2h, NJ), _prep_xT(x2l, NJ)], axis=1)
            )
        else:
            x1T = _prep_xT(x1, NI)
            x2T = _prep_xT(x2, NJ)
        in_maps.append({"x1T": x1T, "x2T": x2T, "Wr": Wr, "biasr": biasr})
    return in_maps


def kernel(input1, input2, W, b):
    from concourse import bass_utils

    # BASS_TRACE needs the axon NTFF hook (antenv.axon_hooks); if the client
    # doesn't ship it, run_bass_kernel_spmd would crash on import. Disable
    # tracing only in that case, and restore the environment afterwards.
    suppress_trace = False
    if os.environ.get("BASS_TRACE"):
        try:
            from antenv.axon_hooks import get_axon_ntff_profile_hook  # noqa: F401
        except Exception:
            suppress_trace = True
    prev = os.environ.get("BASS_NEVER_TRACE")
    if suppress_trace:
        os.environ["BASS_NEVER_TRACE"] = "1"
    try:
        nc = _get_module()
        in_maps = _make_in_maps(input1, input2, W, b)
        res = bass_utils.run_bass_kernel_spmd(
            nc, in_maps, core_ids=list(range(NCORES))
        )
    finally:
        if suppress_trace:
            if prev is None:
                os.environ.pop("BASS_NEVER_TRACE", None)
            else:
                os.environ["BASS_NEVER_TRACE"] = prev
    out = np.empty((4, NJ, NJ, D), dtype=np.float32)
    for c in range(NCORES):
        bb, h = divmod(c, 2)
        out[bb, h * NI : (h + 1) * NI] = res.results[c]["out"]
    return out
